# revision 60
# baseline (speedup 1.0000x reference)
"""GPT forward (8 layers, C=1024, T=1024, B=2, H=16, V=32000) on 8 trn2 cores.

Sharding: TP4 x DP2. Cores 0-3 handle batch 0, cores 4-7 batch 1.
Within a quad, core j owns heads 4j..4j+3, MLP hidden slice j*1024..,
and vocab slice j*8000.. of the LM head.

Device layout: the residual stream lives in SBUF transposed (xT: [C, T],
channels on partitions). All matmuls contract over the partition dim, so
weights (w[C,F] etc.) are natively the stationary lhsT operand and no
activation transposes are ever needed.

The layer body is software-pipelined over two 512-token halves with the
emission order chosen so every LN chain and collective flight is covered
by at least one full PE phase of the other half. LN stats (mean/meansq
over C) are computed on the PE from fp8 copies of x using DoubleRow
matmuls (stats tolerate fp8 easily); LN weight/bias are folded into the
following matmul weights on the host, so the device LN is just
(x-mu)*rstd. Softmax is max-free (logits are provably tiny) with the
denominator fused into the AV matmul via a ones column appended to V.
Matmuls run in bf16 with fp32 PSUM accumulation. Collective payloads and
output logits travel in bf16 (upcast on host). Chunked HBM transfers are
consolidated into single strided DMAs (the HWDGE queue is a serial
resource at ~0.6us per DMA).
"""

import numpy as np
import ml_dtypes

import concourse.bacc as bacc
import concourse.tile as tile
import concourse.mybir as mybir
from concourse import bass_utils

f32 = mybir.dt.float32
bf16 = mybir.dt.bfloat16
fp8 = mybir.dt.float8e4
AF = mybir.ActivationFunctionType
OP = mybir.AluOpType
DR = mybir.MatmulPerfMode.DoubleRow

B, T, C, L, H, F, V = 2, 1024, 1024, 8, 16, 4096, 32000
HD = C // H            # 64
TP = 4                 # tensor-parallel within a quad
HL = H // TP           # 4 local heads
QO = C // TP           # 256 local q/k/v width
FL = F // TP           # 1024 local mlp hidden
VL = V // TP           # 8000 local vocab
NCH = C // 128         # 8 channel chunks
NTC = T // 128         # 8 token chunks
TH = T // 2            # 512 token half
GROUPS = [[0, 1, 2, 3], [4, 5, 6, 7]]
LN_EPS = 1e-5
SCALE = 1.0 / np.sqrt(HD)

_STATE = {}


def _build(collectives=True):
    nc = bacc.Bacc("TRN2", target_bir_lowering=False, debug=False,
                   enable_asserts=False, num_devices=8)

    x0T_d = nc.dram_tensor("x0t", [C, T], f32, kind="ExternalInput").ap()
    wqkv_d = nc.dram_tensor("wqkv", [L, C, 3 * QO], bf16, kind="ExternalInput").ap()
    w1_d = nc.dram_tensor("w1", [L, C, FL], bf16, kind="ExternalInput").ap()
    w2_d = nc.dram_tensor("w2", [L, FL, C], bf16, kind="ExternalInput").ap()
    hw_d = nc.dram_tensor("hw", [C, VL], bf16, kind="ExternalInput").ap()
    # per-partition constant columns (see host packing below)
    bqk_d = nc.dram_tensor("bqk", [128, L * 4], f32, kind="ExternalInput").ap()
    bvb_d = nc.dram_tensor("bvb", [L, 128, QO], bf16, kind="ExternalInput").ap()
    b1_d = nc.dram_tensor("b1c", [128, L * 8], f32, kind="ExternalInput").ap()
    b2_d = nc.dram_tensor("b2c", [128, L * 8], f32, kind="ExternalInput").ap()
    lnfw_d = nc.dram_tensor("lnfw", [128, 8], f32, kind="ExternalInput").ap()
    lnfb_d = nc.dram_tensor("lnfb", [128, 8], f32, kind="ExternalInput").ap()
    mask_d = nc.dram_tensor("mask", [128, 128], bf16, kind="ExternalInput").ap()
    wqs_d = nc.dram_tensor("wqs", [L, 1, 3 * QO], bf16, kind="ExternalInput").ap()
    w1s_d = nc.dram_tensor("w1s", [L, 1, FL], bf16, kind="ExternalInput").ap()
    out_d = nc.dram_tensor("out", [T, VL], bf16, kind="ExternalOutput").ap()

    with tile.TileContext(nc) as tc:
        _prog(nc, tc, x0T_d, wqkv_d, w1_d, w2_d, hw_d, bqk_d, bvb_d, b1_d,
              b2_d, lnfw_d, lnfb_d, mask_d, wqs_d, w1s_d, out_d, collectives)
    nc.compile()
    return nc


def _r8(ap, a=NCH):
    """[a*128, f] dram view -> [128, a, f] (partition-major) for one-shot DMA."""
    return ap.rearrange("(a p) f -> p a f", a=a, p=128)


def _prog(nc, tc, x0T_d, wqkv_d, w1_d, w2_d, hw_d, bqk_d, bvb_d, b1_d, b2_d,
          lnfw_d, lnfb_d, mask_d, wqs_d, w1s_d, out_d, collectives=True):
    import contextlib
    ctx = contextlib.ExitStack()
    with ctx:
        const = ctx.enter_context(tc.tile_pool(name="const", bufs=1))
        xp = ctx.enter_context(tc.tile_pool(name="xres", bufs=NCH))
        hp = ctx.enter_context(tc.tile_pool(name="hln", bufs=17))
        qkp = ctx.enter_context(tc.tile_pool(name="qk", bufs=4))
        vp = ctx.enter_context(tc.tile_pool(name="vsb", bufs=32))
        sbf = ctx.enter_context(tc.tile_pool(name="scrbf", bufs=17))
        sb8 = ctx.enter_context(tc.tile_pool(name="scr8", bufs=3))
        x8p = ctx.enter_context(tc.tile_pool(name="x8", bufs=4))
        bcb = ctx.enter_context(tc.tile_pool(name="bcastb", bufs=2))
        yp = ctx.enter_context(tc.tile_pool(name="ysb", bufs=3))
        sm = ctx.enter_context(tc.tile_pool(name="small", bufs=2))
        smb = ctx.enter_context(tc.tile_pool(name="smallb", bufs=2))
        nmp = ctx.enter_context(tc.tile_pool(name="nmr", bufs=4))
        wqp = ctx.enter_context(tc.tile_pool(name="wqkv", bufs=3))
        w1p = ctx.enter_context(tc.tile_pool(name="w1", bufs=3))
        w2p = ctx.enter_context(tc.tile_pool(name="w2", bufs=2))
        bvp = ctx.enter_context(tc.tile_pool(name="bvb", bufs=2))
        wsp = ctx.enter_context(tc.tile_pool(name="wsum", bufs=2))
        hwp = ctx.enter_context(tc.tile_pool(name="hwsb", bufs=4))
        psb = ctx.enter_context(tc.tile_pool(name="psbig", bufs=6, space="PSUM"))
        pss = ctx.enter_context(tc.tile_pool(name="pssm", bufs=2, space="PSUM"))
        sop = ctx.enter_context(tc.tile_pool(name="sout", bufs=2))
        dr = ctx.enter_context(tc.tile_pool(name="dram", bufs=4, space="DRAM"))

        onesp = const.tile([128, 2, 16], fp8)
        nc.vector.memset(onesp[:], 1.0)
        onesb = const.tile([128, 1], bf16, tag="onesb")
        nc.vector.memset(onesb[:], 1.0)
        eps_t = const.tile([1, 1], f32, tag="eps")
        nc.vector.memset(eps_t[:], LN_EPS)
        mask = const.tile([128, 128], bf16)
        nc.sync.dma_start(mask[:], mask_d[:])
        cols = {}
        for nm, d, w in (("bqk", bqk_d, L * 4), ("b1", b1_d, L * 8),
                         ("b2", b2_d, L * 8)):
            t = const.tile([128, w], f32, tag=f"c_{nm}")
            nc.sync.dma_start(t[:], d[:])
            cols[nm] = t

        # residual stream: 8 persistent fp32 tiles [128 ch, 1024 tok]
        xt = []
        for cc in range(NCH):
            t = xp.tile([128, T], f32)
            nc.sync.dma_start(t[:], x0T_d[cc * 128:(cc + 1) * 128, :])
            xt.append(t)

        def ln_stats(half):
            """fp8 pair copies of x + PE DoubleRow matmuls for mean/meansq.
            The square overwrites the x copy in place after the mean matmul.
            Stats tolerate fp8 easily."""
            hs = slice(half * TH, (half + 1) * TH)
            ssum16 = psb.tile([16, TH], f32, tag="psb")
            sqsum16 = psb.tile([16, TH], f32, tag="psb")
            # ssum: fp8 pair copies + DoubleRow (x fits fp8 range; mean
            # tolerates the quantization). sqsum: bf16 squares (x^2 spans
            # too many decades for fp8).
            for p in range(4):
                xq8 = x8p.tile([128, 2, TH], fp8, tag="x8")
                for j in range(2):
                    cc = 2 * p + j
                    sq = x8p.tile([128, TH], bf16, tag="x8")
                    if cc % 2:
                        nc.scalar.activation(xq8[:, j, :], xt[cc][:, hs],
                                             AF.Copy)
                        nc.vector.tensor_mul(sq[:], xt[cc][:, hs],
                                             xt[cc][:, hs])
                    else:
                        nc.vector.tensor_copy(xq8[:, j, :], xt[cc][:, hs])
                        nc.scalar.activation(sq[:], xt[cc][:, hs], AF.Square)
                    nc.tensor.matmul(sqsum16[0:1, :], onesb[:], sq[:],
                                     start=(cc == 0), stop=(cc == NCH - 1))
                nc.tensor.matmul(ssum16[:], onesp[:], xq8[:, :, :],
                                 start=(p == 0), stop=(p == 3), perf_mode=DR)
            return ssum16[0:1, :], sqsum16[0:1, :]

        def ln_chain(stats):
            ssum, sqsum = stats
            s2 = sm.tile([1, TH], f32, tag="sm")
            nc.scalar.activation(s2[:], ssum[:], AF.Square)
            varu = sm.tile([1, TH], f32, tag="sm")
            nc.vector.scalar_tensor_tensor(varu[:], s2[:], -1.0 / C, sqsum[:],
                                           op0=OP.mult, op1=OP.add)
            std = sm.tile([1, TH], f32, tag="sm")
            nc.scalar.activation(std[:], varu[:], AF.Sqrt,
                                 scale=float(1.0 / C), bias=eps_t[:])
            rstd = smb.tile([1, TH], bf16, tag="smb")
            with nc.allow_low_precision(reason="rstd bf16 feeds bf16 matmul"):
                nc.vector.reciprocal(rstd[:], std[:])
            nmr = nmp.tile([1, TH], bf16, tag="nm")
            nc.vector.scalar_tensor_tensor(nmr[:], ssum[:], -1.0 / C, rstd[:],
                                           op0=OP.mult, op1=OP.mult)
            rstd_b = bcb.tile([128, TH], bf16, tag="bcb")
            nc.gpsimd.partition_broadcast(rstd_b[:], rstd[:])
            return rstd_b, nmr

        def ln_norm(half, chain, final=False):
            rstd_b, nmr = chain
            hs = slice(half * TH, (half + 1) * TH)
            nmr_b = None
            if final:
                nmr_b = bcb.tile([128, TH], bf16, tag="bcb")
                nc.gpsimd.partition_broadcast(nmr_b[:], nmr[:])
            out = []
            for cc in range(NCH):
                h = hp.tile([128, TH], bf16)
                eng = nc.gpsimd if cc % 2 else nc.vector
                eng.tensor_mul(h[:], xt[cc][:, hs], rstd_b[:])
                if final:
                    nc.vector.tensor_add(h[:], h[:], nmr_b[:])
                out.append(h)
            return out, nmr

        def ln_post(half, stats, final=False):
            """Stats chain + broadcast + normalize: no PE work, so this can
            be emitted anywhere without blocking the PE queue."""
            hs = slice(half * TH, (half + 1) * TH)
            ssum, sqsum = stats
            # rstd = 1/sqrt((sqsum - ssum^2/C)/C + eps); nmr = -ssum/C * rstd
            s2 = sm.tile([1, TH], f32, tag="sm")
            nc.scalar.activation(s2[:], ssum[:], AF.Square)
            varu = sm.tile([1, TH], f32, tag="sm")
            nc.vector.scalar_tensor_tensor(varu[:], s2[:], -1.0 / C, sqsum[:],
                                           op0=OP.mult, op1=OP.add)
            std = sm.tile([1, TH], f32, tag="sm")
            nc.scalar.activation(std[:], varu[:], AF.Sqrt,
                                 scale=float(1.0 / C), bias=eps_t[:])
            rstd = smb.tile([1, TH], bf16, tag="smb")
            with nc.allow_low_precision(reason="rstd bf16 feeds bf16 matmul"):
                nc.vector.reciprocal(rstd[:], std[:])
            nmr = nmp.tile([1, TH], bf16, tag="nm")
            nc.vector.scalar_tensor_tensor(nmr[:], ssum[:], -1.0 / C, rstd[:],
                                           op0=OP.mult, op1=OP.mult)
            rstd_b = bcb.tile([128, TH], bf16, tag="bcb")
            nc.gpsimd.partition_broadcast(rstd_b[:], rstd[:])
            nmr_b = None
            if final:
                nmr_b = bcb.tile([128, TH], bf16, tag="bcb")
                nc.gpsimd.partition_broadcast(nmr_b[:], nmr[:])
            out = []
            for cc in range(NCH):
                h = hp.tile([128, TH], bf16)
                eng = nc.gpsimd if cc % 2 else nc.vector
                eng.tensor_mul(h[:], xt[cc][:, hs], rstd_b[:])
                if final:
                    nc.vector.tensor_add(h[:], h[:], nmr_b[:])
                out.append(h)
            return out, nmr

        def layernorm(half, final=False):
            return ln_post(half, ln_stats(half), final)[0]

        def ar_read(l, r_in, r_out, half):
            """Read back MLP allreduce for (l, half), add into residual."""
            hs = slice(half * TH, (half + 1) * TH)
            rt = sb8.tile([128, NCH, TH], bf16, tag="sb8")
            src_t = r_out if collectives != "skip" else r_in
            for g in range(2):
                nc.sync.dma_start(rt[:, 4 * g:4 * g + 4, :],
                                  _r8(src_t[g * 512:(g + 1) * 512, :], a=4))
                for cc in range(4 * g, 4 * g + 4):
                    eng = nc.gpsimd if cc % 2 else nc.vector
                    eng.tensor_add(xt[cc][:, hs], xt[cc][:, hs], rt[:, cc, :])

        def _load_half(pool, dram, width):
            ts = []
            for g in range(2):
                t = pool.tile([128, 4, width], bf16)
                nc.sync.dma_start(
                    t[:], _r8(dram[g * 512:(g + 1) * 512, :], a=4))
                ts.append(t)
            return ts

        def load_wq(l):
            t = _load_half(wqp, wqkv_d[l], 3 * QO)
            bvt = bvp.tile([128, QO], bf16)
            nc.sync.dma_start(bvt[:], bvb_d[l, :, :])
            ws = wsp.tile([1, 3 * QO], bf16, tag="wqs")
            nc.sync.dma_start(ws[:], wqs_d[l])
            return t, bvt, ws

        def load_w1(l):
            t = _load_half(w1p, w1_d[l], FL)
            ws = wsp.tile([1, FL], bf16, tag="w1s")
            nc.sync.dma_start(ws[:], w1s_d[l])
            return t, ws

        def load_w2(l):
            return _load_half(w2p, w2_d[l], C)

        ar_bufs = [None, None]
        head_pre = []

        # LM head helpers (normal orientation: out[tok, vocab])
        NVB = (VL + 511) // 512

        def head_load(vb):
            vn = min(512, VL - vb * 512)
            rhs_t = []
            for g in range(2):
                wt = hwp.tile([128, 4, 512], bf16)
                nc.sync.dma_start(
                    wt[:, :, 0:vn],
                    _r8(hw_d[g * 512:(g + 1) * 512,
                             vb * 512:vb * 512 + vn], a=4))
                rhs_t.append(wt)
            return rhs_t

        def head_block(vb, rhs_t, tccs):
            vn = min(512, VL - vb * 512)
            for tcc in tccs:
                to = (tcc % 4) * 128
                ph = psb.tile([128, 512], f32, tag="psb")
                for cc in range(NCH):
                    nc.tensor.matmul(ph[:, 0:vn],
                                     hf[tcc // 4][cc][:, to:to + 128],
                                     rhs_t[cc // 4][:, cc % 4, 0:vn],
                                     start=(cc == 0), stop=(cc == NCH - 1))
                so = sop.tile([128, 512], bf16, tag="so")
                if tcc % 2:
                    nc.vector.tensor_copy(so[:, 0:vn], ph[:, 0:vn])
                else:
                    nc.scalar.activation(so[:, 0:vn], ph[:, 0:vn], AF.Copy)
                nc.sync.dma_start(out_d[tcc * 128:(tcc + 1) * 128,
                                        vb * 512:vb * 512 + vn],
                                  so[:, 0:vn])


        # prologue: weights for layer 0 + LN1 of half 0
        wq_cur = load_wq(0)
        w1_cur = load_w1(0)
        w2_cur = load_w2(0)
        h1a_nm = ln_post(0, ln_stats(0))

        for l in range(L):
            (wq3, bvt, wqs), (w13, w1s), w23 = wq_cur, w1_cur, w2_cur

            qk_t = [qkp.tile([128, T], bf16, tag="qkt", name=f"qk{l}_{i}")
                    for i in range(4)]
            v_t = [[None] * HL for _ in range(NTC)]
            ag_bufs = [None, None]

            def qkv_qk(half, h1, nmr):
                hs = slice(half * TH, (half + 1) * TH)
                for oc in range(4):
                    p = psb.tile([128, TH], f32, tag="psb")
                    for cc in range(NCH):
                        nc.tensor.matmul(p[:],
                                         wq3[cc // 4][:, cc % 4,
                                                      oc * 128:(oc + 1) * 128],
                                         h1[cc][:],
                                         start=(cc == 0), stop=False)
                    nc.tensor.matmul(p[:],
                                     wqs[0:1, oc * 128:(oc + 1) * 128],
                                     nmr[:], start=False, stop=True)
                    if oc % 2:
                        nc.scalar.activation(
                            qk_t[oc][:, hs], p[:], AF.Identity,
                            bias=cols["bqk"][:, l * 4 + oc:l * 4 + oc + 1])
                    else:
                        nc.vector.tensor_scalar_add(
                            qk_t[oc][:, hs], p[:],
                            cols["bqk"][:, l * 4 + oc:l * 4 + oc + 1])

            def qkv_v(half, h1, nmr):
                # v chunks of this half (ones column fused for softmax denom)
                for tcc in range(4 * half, 4 * half + 4):
                    to = tcc * 128 - half * TH
                    pv = pss.tile([128, QO], f32, tag="pss")
                    for cc in range(NCH):
                        nc.tensor.matmul(pv[:], h1[cc][:, to:to + 128],
                                         wq3[cc // 4][:, cc % 4,
                                                      2 * QO:3 * QO],
                                         start=(cc == 0), stop=False)
                    nc.tensor.matmul(pv[:], nmr[0:1, to:to + 128],
                                     wqs[0:1, 2 * QO:3 * QO],
                                     start=False, stop=True)
                    for hh in range(HL):
                        vt = vp.tile([128, HD + 1], bf16)
                        nc.vector.memset(vt[:, HD:HD + 1], 1.0)
                        nc.vector.tensor_add(vt[:, 0:HD],
                                             pv[:, hh * HD:(hh + 1) * HD],
                                             bvt[:, hh * HD:(hh + 1) * HD])
                        v_t[tcc][hh] = vt

            def attn(half, mid=None, mid_at=1):
                """Head-pipelined attention + AllGather launch for a half."""
                y_sb = [yp.tile([128, TH], bf16, tag="y",
                                name=f"ysb{l}_{half}_{i}") for i in range(2)]
                nsi = 4 * half + 4

                def scores(hh):
                    qi, ro = hh // 2, (hh % 2) * 64
                    att = []
                    for si in range(nsi):
                        pa = psb.tile([128, TH], f32, tag="psb")
                        lhs = qk_t[2 + qi][ro:ro + 64,
                                           si * 128:(si + 1) * 128]
                        sc = max(si * 128 - half * TH, 0)
                        nc.tensor.matmul(pa[:, sc:TH], lhs,
                                         qk_t[qi][ro:ro + 64,
                                                  half * TH + sc:
                                                  (half + 1) * TH],
                                         start=True, stop=True)
                        ab = sbf.tile([128, TH], bf16, tag="sbf")
                        if sc:
                            nc.vector.memset(ab[:, 0:sc], 0.0)
                        nc.scalar.activation(ab[:, sc:TH], pa[:, sc:TH],
                                             AF.Exp, scale=float(SCALE))
                        if si >= 4 * half:  # diagonal block: causal mask
                            nc.vector.tensor_mul(ab[:, sc:sc + 128],
                                                 ab[:, sc:sc + 128], mask[:])
                        att.append(ab)
                    return att

                def av(hh, att):
                    py = pss.tile([HD + 1, TH], f32, tag="pss")
                    for qb in range(4):
                        qs = slice(qb * 128, (qb + 1) * 128)
                        last = 4 * half + qb
                        for si in range(last + 1):
                            nc.tensor.matmul(py[:, qs], v_t[si][hh][:],
                                             att[si][:, qs],
                                             start=(si == 0),
                                             stop=(si == last))
                    den_r = smb.tile([1, TH], bf16, tag="smb")
                    with nc.allow_low_precision(reason="softmax denom bf16"):
                        nc.vector.reciprocal(den_r[:], py[HD:HD + 1, :])
                    den_b = bcb.tile([64, TH], bf16, tag="bcb")
                    nc.gpsimd.partition_broadcast(den_b[:], den_r[:])
                    nc.vector.tensor_mul(
                        y_sb[hh // 2][(hh % 2) * 64:(hh % 2) * 64 + 64, :],
                        py[0:HD, :], den_b[:])

                prev = None
                for hh in range(HL):
                    att = scores(hh)
                    if prev is not None:
                        av(*prev)
                        if hh == mid_at and mid is not None:
                            mid()
                    prev = (hh, att)
                av(*prev)

                g_in = dr.tile([QO, TH], bf16, tag="gin")
                for i in range(2):
                    nc.sync.dma_start(g_in[i * 128:(i + 1) * 128, :],
                                      y_sb[i][:])
                g_out = dr.tile([C, TH], bf16, tag="gout")
                if collectives is True:
                    nc.gpsimd.collective_compute(
                        "AllGather", OP.bypass, replica_groups=GROUPS,
                        ins=[g_in.opt()], outs=[g_out.opt()])
                elif collectives == "local":
                    for q in range(TP):
                        nc.sync.dma_start(g_out[q * QO:(q + 1) * QO, :],
                                          g_in[:])
                ag_bufs[half] = (g_in, g_out)

            def ag_read(half):
                hs = slice(half * TH, (half + 1) * TH)
                g_in, g_out = ag_bufs[half]
                yt = sb8.tile([128, NCH, TH], bf16, tag="sb8")
                for g in range(2):
                    if collectives != "skip":
                        nc.sync.dma_start(
                            yt[:, 4 * g:4 * g + 4, :],
                            _r8(g_out[g * 512:(g + 1) * 512, :], a=4))
                    else:
                        for cc in range(4 * g, 4 * g + 4):
                            nc.sync.dma_start(
                                yt[:, cc, :],
                                g_in[(cc % 2) * 128:(cc % 2) * 128 + 128, :])
                    for cc in range(4 * g, 4 * g + 4):
                        eng = nc.gpsimd if cc % 2 else nc.vector
                        eng.tensor_add(xt[cc][:, hs], xt[cc][:, hs],
                                       yt[:, cc, :])

            def mlp1(half, h2, nmr2):
                a_t = []
                for fc in range(NCH):
                    pm = psb.tile([128, TH], f32, tag="psb")
                    for cc in range(NCH):
                        nc.tensor.matmul(pm[:],
                                         w13[cc // 4][:, cc % 4, fc * 128:(fc + 1) * 128],
                                         h2[cc][:],
                                         start=(cc == 0), stop=False)
                    nc.tensor.matmul(pm[:],
                                     w1s[0:1, fc * 128:(fc + 1) * 128],
                                     nmr2[:], start=False, stop=True)
                    ga = sbf.tile([128, TH], bf16, tag="sbf")
                    nc.scalar.activation(
                        ga[:], pm[:], AF.Gelu,
                        bias=cols["b1"][:, l * 8 + fc:l * 8 + fc + 1])
                    a_t.append(ga)
                return a_t

            def mlp2(half, a_t):
                mo = sb8.tile([128, NCH, TH], bf16, tag="sb8")
                for cc in range(NCH):
                    pm2 = psb.tile([128, TH], f32, tag="psb")
                    for fc in range(NCH):
                        nc.tensor.matmul(pm2[:],
                                         w23[fc // 4][:, fc % 4, cc * 128:(cc + 1) * 128],
                                         a_t[fc][:],
                                         start=(fc == 0), stop=(fc == NCH - 1))
                    b2c = cols["b2"][:, l * 8 + cc:l * 8 + cc + 1]
                    if cc % 2:
                        nc.vector.tensor_scalar_add(mo[:, cc, :], pm2[:], b2c)
                    else:
                        nc.scalar.activation(mo[:, cc, :], pm2[:],
                                             AF.Identity, bias=b2c)
                r_in = dr.tile([C, TH], bf16, tag="rin")
                r_out = dr.tile([C, TH], bf16, tag="rout")
                for g in range(2):
                    gs = slice(g * 512, (g + 1) * 512)
                    nc.sync.dma_start(_r8(r_in[gs, :], a=4),
                                      mo[:, 4 * g:4 * g + 4, :])
                    if collectives is True:
                        nc.gpsimd.collective_compute(
                            "AllReduce", OP.add, replica_groups=GROUPS,
                            ins=[r_in[gs, :].opt()], outs=[r_out[gs, :].opt()])
                    elif collectives == "local":
                        nc.sync.dma_start(r_out[gs, :], r_in[gs, :])
                ar_bufs[half] = (r_in, r_out)

            # schedule: LN chains / collective flights hide behind the other
            # half's PE phases; ln_stats PE matmuls are placed right after
            # phases that give their input dependencies time to resolve
            mid_state = {}

            h1a, nm1a = h1a_nm
            qkv_qk(0, h1a, nm1a)
            if ar_bufs[1] is not None:
                ar_read(l - 1, *ar_bufs[1], 1)
            qkv_v(0, h1a, nm1a)

            def mid0():
                mid_state["c1b"] = ln_chain(ln_stats(1))

            attn(0, mid=mid0, mid_at=2)
            h1b, nm1b = ln_norm(1, mid_state["c1b"])
            wq_cur = load_wq(l + 1) if l + 1 < L else None
            qkv_qk(1, h1b, nm1b)
            qkv_v(1, h1b, nm1b)
            ag_read(0)

            def mid1():
                mid_state["c2a"] = ln_chain(ln_stats(0))

            attn(1, mid=mid1)
            h2a, nm2a = ln_norm(0, mid_state["c2a"])
            w1_cur = load_w1(l + 1) if l + 1 < L else None
            a0 = mlp1(0, h2a, nm2a)
            ag_read(1)
            st2b = ln_stats(1)
            h2b, nm2b = ln_post(1, st2b)
            mlp2(0, a0)
            w2_cur = load_w2(l + 1) if l + 1 < L else None
            if l == L - 1:
                head_pre.append(head_load(0))
                head_pre.append(head_load(1))
            a1 = mlp1(1, h2b, nm2b)
            ar_read(l, *ar_bufs[0], 0)
            st1a = ln_stats(0)
            h1a_nm = ln_post(0, st1a, final=(l == L - 1))
            mlp2(1, a1)

        # first two vocab blocks' half-0 tokens cover the final LN of half 1
        hf = [h1a_nm[0], None]
        rhs0, rhs1 = head_pre
        head_block(0, rhs0, range(4))
        ar_read(L - 1, *ar_bufs[1], 1)
        stf = ln_stats(1)
        head_block(1, rhs1, range(4))
        hf[1] = ln_post(1, stf, final=True)[0]
        head_block(0, rhs0, range(4, NTC))
        head_block(1, rhs1, range(4, NTC))
        for vb in range(2, NVB):
            rhs_t = head_load(vb)
            head_block(vb, rhs_t, range(NTC))

def _prep_inputs(idx, tok_emb, pos_emb, ln1_w, ln1_b, wq, bq, wk, bk, wv, bv,
                 ln2_w, ln2_b, w1, b1, w2, b2, lnf_w, lnf_b, head_w):
    bf = ml_dtypes.bfloat16

    def cols128(a):  # [L, C] -> [128, L*8] per-partition column packing
        a = np.ascontiguousarray(a, np.float32)
        Lx = a.shape[0]
        return a.reshape(Lx, NCH, 128).transpose(2, 0, 1).reshape(128, Lx * NCH)

    # fold LN affine into the consuming projections:
    #   q = ((x-mu)*rstd) @ (ln1_w * wq) + (bq + ln1_b @ wq), etc.
    wq = np.asarray(wq, np.float32)
    wk = np.asarray(wk, np.float32)
    wv = np.asarray(wv, np.float32)
    w1 = np.asarray(w1, np.float32)
    bq = np.asarray(bq, np.float32) + np.einsum("lc,lcf->lf", ln1_b, wq)
    bk = np.asarray(bk, np.float32) + np.einsum("lc,lcf->lf", ln1_b, wk)
    bv = np.asarray(bv, np.float32) + np.einsum("lc,lcf->lf", ln1_b, wv)
    b1 = np.asarray(b1, np.float32) + np.einsum("lc,lcf->lf", ln2_b, w1)
    wq = ln1_w[:, :, None] * wq
    wk = ln1_w[:, :, None] * wk
    wv = ln1_w[:, :, None] * wv
    w1 = ln2_w[:, :, None] * w1

    mask = np.zeros((128, 128), np.float32)
    p, t = np.meshgrid(np.arange(128), np.arange(128), indexing="ij")
    mask[p <= t] = 1.0
    in_maps = []
    shard_cache = {}
    x0s = [np.ascontiguousarray(
        (tok_emb[np.asarray(idx[g], np.int64)] + pos_emb[0]).T, np.float32)
        for g in range(B)]
    for c in range(8):
        g, j = c // 4, c % 4
        if j in shard_cache:
            m = dict(shard_cache[j])
            m["x0t"] = x0s[g]
            in_maps.append(m)
            continue
        m = {
            "wqkv": np.ascontiguousarray(np.concatenate(
                [wq[:, :, j * QO:(j + 1) * QO], wk[:, :, j * QO:(j + 1) * QO],
                 wv[:, :, j * QO:(j + 1) * QO]], axis=2)).astype(bf),
            "w1": np.ascontiguousarray(w1[:, :, j * FL:(j + 1) * FL]).astype(bf),
            "w2": np.ascontiguousarray(w2[:, j * FL:(j + 1) * FL, :]).astype(bf),
            "hw": np.ascontiguousarray(
                lnf_w[:, None] * head_w[:, j * VL:(j + 1) * VL]).astype(bf),

            "bqk": np.ascontiguousarray(np.stack(
                [bq[:, j * QO:(j + 1) * QO].reshape(L, 2, 128),
                 bk[:, j * QO:(j + 1) * QO].reshape(L, 2, 128)],
                axis=1).reshape(L * 4, 128).T, np.float32),
            "bvb": np.ascontiguousarray(np.broadcast_to(
                bv[:, None, j * QO:(j + 1) * QO],
                (L, 128, QO)).astype(bf)),
            "wqs": np.concatenate(
                [wq[:, :, j * QO:(j + 1) * QO].sum(axis=1),
                 wk[:, :, j * QO:(j + 1) * QO].sum(axis=1),
                 wv[:, :, j * QO:(j + 1) * QO].sum(axis=1)],
                axis=1)[:, None, :].astype(bf),
            "w1s": w1[:, :, j * FL:(j + 1) * FL].sum(axis=1)[:, None, :]
                .astype(bf),
            "b1c": cols128(b1[:, j * FL:(j + 1) * FL]),
            "b2c": cols128(b2 if j == 0 else np.zeros_like(
                np.asarray(b2, np.float32))),
            "lnfw": cols128(lnf_w[None]), "lnfb": cols128(lnf_b[None]),
            "mask": mask.astype(bf),
        }
        m["x0t"] = x0s[g]
        shard_cache[j] = m
        in_maps.append(m)
    return in_maps


def kernel(**inputs):
    if "nc" not in _STATE:
        _STATE["nc"] = _build()
    nc = _STATE["nc"]
    ins = {k: np.asarray(v) for k, v in inputs.items()}
    in_maps = _prep_inputs(**ins)
    res = bass_utils.run_bass_kernel_spmd(nc, in_maps, core_ids=list(range(8)))
    outs = res.results
    # lnf_b's contribution to the logits is a per-vocab constant, added here
    hb = (np.asarray(ins["lnf_b"], np.float32)
          @ np.asarray(ins["head_w"], np.float32))
    full = np.empty((B, T, V), np.float32)
    for c in range(8):
        g, j = c // 4, c % 4
        full[g, :, j * VL:(j + 1) * VL] = (
            np.asarray(outs[c]["out"], np.float32) + hb[j * VL:(j + 1) * VL])
    return full


# revision 67
# speedup vs baseline: 1.0308x; 1.0308x over previous
"""GPT forward (8 layers, C=1024, T=1024, B=2, H=16, V=32000) on 8 trn2 cores.

Sharding: TP4 x DP2. Cores 0-3 handle batch 0, cores 4-7 batch 1.
Within a quad, core j owns heads 4j..4j+3, MLP hidden slice j*1024..,
and vocab slice j*8000.. of the LM head.

Device layout: the residual stream lives in SBUF transposed (xT: [C, T],
channels on partitions). All matmuls contract over the partition dim, so
weights (w[C,F] etc.) are natively the stationary lhsT operand and no
activation transposes are ever needed.

The layer body is software-pipelined over two 512-token halves with the
emission order chosen so every LN chain and collective flight is covered
by at least one full PE phase of the other half. LN stats (mean/meansq
over C) are computed on the PE from fp8 copies of x using DoubleRow
matmuls (stats tolerate fp8 easily); LN weight/bias are folded into the
following matmul weights on the host, so the device LN is just
(x-mu)*rstd. Softmax is max-free (logits are provably tiny) with the
denominator fused into the AV matmul via a ones column appended to V.
Matmuls run in bf16 with fp32 PSUM accumulation. Collective payloads and
output logits travel in bf16 (upcast on host). Chunked HBM transfers are
consolidated into single strided DMAs (the HWDGE queue is a serial
resource at ~0.6us per DMA).
"""

import numpy as np
import ml_dtypes

import concourse.bacc as bacc
import concourse.tile as tile
import concourse.mybir as mybir
from concourse import bass_utils

f32 = mybir.dt.float32
bf16 = mybir.dt.bfloat16
fp8 = mybir.dt.float8e4
AF = mybir.ActivationFunctionType
OP = mybir.AluOpType
DR = mybir.MatmulPerfMode.DoubleRow

B, T, C, L, H, F, V = 2, 1024, 1024, 8, 16, 4096, 32000
HD = C // H            # 64
TP = 4                 # tensor-parallel within a quad
HL = H // TP           # 4 local heads
QO = C // TP           # 256 local q/k/v width
FL = F // TP           # 1024 local mlp hidden
VL = V // TP           # 8000 local vocab
NCH = C // 128         # 8 channel chunks
NTC = T // 128         # 8 token chunks
TH = T // 2            # 512 token half
GROUPS = [[0, 1, 2, 3], [4, 5, 6, 7]]
LN_EPS = 1e-5
SCALE = 1.0 / np.sqrt(HD)

_STATE = {}


def _build(collectives=True):
    nc = bacc.Bacc("TRN2", target_bir_lowering=False, debug=False,
                   enable_asserts=False, num_devices=8)

    x0T_d = nc.dram_tensor("x0t", [C, T], f32, kind="ExternalInput").ap()
    wqkv_d = nc.dram_tensor("wqkv", [L, C, 3 * QO], bf16, kind="ExternalInput").ap()
    w1_d = nc.dram_tensor("w1", [L, C, FL], bf16, kind="ExternalInput").ap()
    w2_d = nc.dram_tensor("w2", [L, FL, C], bf16, kind="ExternalInput").ap()
    hw_d = nc.dram_tensor("hw", [C, VL], bf16, kind="ExternalInput").ap()
    # per-partition constant columns (see host packing below)
    bqk_d = nc.dram_tensor("bqk", [128, L * 4], f32, kind="ExternalInput").ap()
    bvb_d = nc.dram_tensor("bvb", [L, 128, QO], bf16, kind="ExternalInput").ap()
    b1_d = nc.dram_tensor("b1c", [128, L * 8], f32, kind="ExternalInput").ap()
    b2_d = nc.dram_tensor("b2c", [128, L * 8], f32, kind="ExternalInput").ap()
    lnfw_d = nc.dram_tensor("lnfw", [128, 8], f32, kind="ExternalInput").ap()
    lnfb_d = nc.dram_tensor("lnfb", [128, 8], f32, kind="ExternalInput").ap()
    mask_d = nc.dram_tensor("mask", [128, 128], bf16, kind="ExternalInput").ap()
    wqs_d = nc.dram_tensor("wqs", [L, 1, 3 * QO], bf16, kind="ExternalInput").ap()
    w1s_d = nc.dram_tensor("w1s", [L, 1, FL], bf16, kind="ExternalInput").ap()
    out_d = nc.dram_tensor("out", [T, VL], bf16, kind="ExternalOutput").ap()

    with tile.TileContext(nc) as tc:
        _prog(nc, tc, x0T_d, wqkv_d, w1_d, w2_d, hw_d, bqk_d, bvb_d, b1_d,
              b2_d, lnfw_d, lnfb_d, mask_d, wqs_d, w1s_d, out_d, collectives)
    nc.compile()
    return nc


def _r8(ap, a=NCH):
    """[a*128, f] dram view -> [128, a, f] (partition-major) for one-shot DMA."""
    return ap.rearrange("(a p) f -> p a f", a=a, p=128)


def _prog(nc, tc, x0T_d, wqkv_d, w1_d, w2_d, hw_d, bqk_d, bvb_d, b1_d, b2_d,
          lnfw_d, lnfb_d, mask_d, wqs_d, w1s_d, out_d, collectives=True):
    import contextlib
    ctx = contextlib.ExitStack()
    with ctx:
        const = ctx.enter_context(tc.tile_pool(name="const", bufs=1))
        xp = ctx.enter_context(tc.tile_pool(name="xres", bufs=NCH))
        hp = ctx.enter_context(tc.tile_pool(name="hln", bufs=17))
        qkp = ctx.enter_context(tc.tile_pool(name="qk", bufs=4))
        vp = ctx.enter_context(tc.tile_pool(name="vsb", bufs=32))
        sbf = ctx.enter_context(tc.tile_pool(name="scrbf", bufs=17))
        sb8 = ctx.enter_context(tc.tile_pool(name="scr8", bufs=3))
        x8p = ctx.enter_context(tc.tile_pool(name="x8", bufs=4))
        bcb = ctx.enter_context(tc.tile_pool(name="bcastb", bufs=2))
        yp = ctx.enter_context(tc.tile_pool(name="ysb", bufs=3))
        sm = ctx.enter_context(tc.tile_pool(name="small", bufs=2))
        smb = ctx.enter_context(tc.tile_pool(name="smallb", bufs=2))
        nmp = ctx.enter_context(tc.tile_pool(name="nmr", bufs=4))
        wqp = ctx.enter_context(tc.tile_pool(name="wqkv", bufs=3))
        w1p = ctx.enter_context(tc.tile_pool(name="w1", bufs=3))
        w2p = ctx.enter_context(tc.tile_pool(name="w2", bufs=2))
        bvp = ctx.enter_context(tc.tile_pool(name="bvb", bufs=2))
        wsp = ctx.enter_context(tc.tile_pool(name="wsum", bufs=2))
        hwp = ctx.enter_context(tc.tile_pool(name="hwsb", bufs=4))
        psb = ctx.enter_context(tc.tile_pool(name="psbig", bufs=6, space="PSUM"))
        pss = ctx.enter_context(tc.tile_pool(name="pssm", bufs=2, space="PSUM"))
        sop = ctx.enter_context(tc.tile_pool(name="sout", bufs=2))
        dr = ctx.enter_context(tc.tile_pool(name="dram", bufs=4, space="DRAM"))

        onesp = const.tile([128, 2, 16], fp8)
        nc.vector.memset(onesp[:], 1.0)
        onesb = const.tile([128, 1], bf16, tag="onesb")
        nc.vector.memset(onesb[:], 1.0)
        eps_t = const.tile([1, 1], f32, tag="eps")
        nc.vector.memset(eps_t[:], LN_EPS)
        mask = const.tile([128, 128], bf16)
        nc.sync.dma_start(mask[:], mask_d[:])
        cols = {}
        for nm, d, w in (("bqk", bqk_d, L * 4), ("b1", b1_d, L * 8),
                         ("b2", b2_d, L * 8)):
            t = const.tile([128, w], f32, tag=f"c_{nm}")
            nc.sync.dma_start(t[:], d[:])
            cols[nm] = t

        # residual stream: 8 persistent fp32 tiles [128 ch, 1024 tok]
        xt = []
        for cc in range(NCH):
            t = xp.tile([128, T], f32)
            nc.sync.dma_start(t[:], x0T_d[cc * 128:(cc + 1) * 128, :])
            xt.append(t)

        def ln_stats(half):
            """fp8 pair copies of x + PE DoubleRow matmuls for mean/meansq.
            The square overwrites the x copy in place after the mean matmul.
            Stats tolerate fp8 easily."""
            hs = slice(half * TH, (half + 1) * TH)
            ssum16 = psb.tile([16, TH], f32, tag="psb")
            sqsum16 = psb.tile([16, TH], f32, tag="psb")
            # ssum: fp8 pair copies + DoubleRow (x fits fp8 range; mean
            # tolerates the quantization). sqsum: bf16 squares (x^2 spans
            # too many decades for fp8).
            for p in range(4):
                xq8 = x8p.tile([128, 2, TH], fp8, tag="x8")
                for j in range(2):
                    cc = 2 * p + j
                    sq = x8p.tile([128, TH], bf16, tag="x8")
                    if cc % 2:
                        nc.scalar.activation(xq8[:, j, :], xt[cc][:, hs],
                                             AF.Copy)
                        nc.vector.tensor_mul(sq[:], xt[cc][:, hs],
                                             xt[cc][:, hs])
                    else:
                        nc.vector.tensor_copy(xq8[:, j, :], xt[cc][:, hs])
                        nc.scalar.activation(sq[:], xt[cc][:, hs], AF.Square)
                    nc.tensor.matmul(sqsum16[0:1, :], onesb[:], sq[:],
                                     start=(cc == 0), stop=(cc == NCH - 1))
                nc.tensor.matmul(ssum16[:], onesp[:], xq8[:, :, :],
                                 start=(p == 0), stop=(p == 3), perf_mode=DR)
            return ssum16[0:1, :], sqsum16[0:1, :]

        def ln_chain(stats):
            ssum, sqsum = stats
            s2 = sm.tile([1, TH], f32, tag="sm")
            nc.scalar.activation(s2[:], ssum[:], AF.Square)
            varu = sm.tile([1, TH], f32, tag="sm")
            nc.vector.scalar_tensor_tensor(varu[:], s2[:], -1.0 / C, sqsum[:],
                                           op0=OP.mult, op1=OP.add)
            std = sm.tile([1, TH], f32, tag="sm")
            nc.scalar.activation(std[:], varu[:], AF.Sqrt,
                                 scale=float(1.0 / C), bias=eps_t[:])
            rstd = smb.tile([1, TH], bf16, tag="smb")
            with nc.allow_low_precision(reason="rstd bf16 feeds bf16 matmul"):
                nc.vector.reciprocal(rstd[:], std[:])
            nmr = nmp.tile([1, TH], bf16, tag="nm")
            nc.vector.scalar_tensor_tensor(nmr[:], ssum[:], -1.0 / C, rstd[:],
                                           op0=OP.mult, op1=OP.mult)
            rstd_b = bcb.tile([128, TH], bf16, tag="bcb")
            nc.gpsimd.partition_broadcast(rstd_b[:], rstd[:])
            return rstd_b, nmr

        def ln_norm(half, chain, final=False):
            rstd_b, nmr = chain
            hs = slice(half * TH, (half + 1) * TH)
            nmr_b = None
            if final:
                nmr_b = bcb.tile([128, TH], bf16, tag="bcb")
                nc.gpsimd.partition_broadcast(nmr_b[:], nmr[:])
            out = []
            for cc in range(NCH):
                h = hp.tile([128, TH], bf16)
                eng = nc.gpsimd if cc % 2 else nc.vector
                eng.tensor_mul(h[:], xt[cc][:, hs], rstd_b[:])
                if final:
                    nc.vector.tensor_add(h[:], h[:], nmr_b[:])
                out.append(h)
            return out, nmr

        def ln_post(half, stats, final=False):
            """Stats chain + broadcast + normalize: no PE work, so this can
            be emitted anywhere without blocking the PE queue."""
            hs = slice(half * TH, (half + 1) * TH)
            ssum, sqsum = stats
            # rstd = 1/sqrt((sqsum - ssum^2/C)/C + eps); nmr = -ssum/C * rstd
            s2 = sm.tile([1, TH], f32, tag="sm")
            nc.scalar.activation(s2[:], ssum[:], AF.Square)
            varu = sm.tile([1, TH], f32, tag="sm")
            nc.vector.scalar_tensor_tensor(varu[:], s2[:], -1.0 / C, sqsum[:],
                                           op0=OP.mult, op1=OP.add)
            std = sm.tile([1, TH], f32, tag="sm")
            nc.scalar.activation(std[:], varu[:], AF.Sqrt,
                                 scale=float(1.0 / C), bias=eps_t[:])
            rstd = smb.tile([1, TH], bf16, tag="smb")
            with nc.allow_low_precision(reason="rstd bf16 feeds bf16 matmul"):
                nc.vector.reciprocal(rstd[:], std[:])
            nmr = nmp.tile([1, TH], bf16, tag="nm")
            nc.vector.scalar_tensor_tensor(nmr[:], ssum[:], -1.0 / C, rstd[:],
                                           op0=OP.mult, op1=OP.mult)
            rstd_b = bcb.tile([128, TH], bf16, tag="bcb")
            nc.gpsimd.partition_broadcast(rstd_b[:], rstd[:])
            nmr_b = None
            if final:
                nmr_b = bcb.tile([128, TH], bf16, tag="bcb")
                nc.gpsimd.partition_broadcast(nmr_b[:], nmr[:])
            out = []
            for cc in range(NCH):
                h = hp.tile([128, TH], bf16)
                eng = nc.gpsimd if cc % 2 else nc.vector
                eng.tensor_mul(h[:], xt[cc][:, hs], rstd_b[:])
                if final:
                    nc.vector.tensor_add(h[:], h[:], nmr_b[:])
                out.append(h)
            return out, nmr

        def layernorm(half, final=False):
            return ln_post(half, ln_stats(half), final)[0]

        def ar_read(l, r_in, r_out, half):
            """Read back MLP allreduce for (l, half), add into residual."""
            hs = slice(half * TH, (half + 1) * TH)
            rt = sb8.tile([128, NCH, TH], bf16, tag="sb8")
            src_t = r_out if collectives != "skip" else r_in
            for g in range(4):
                nc.sync.dma_start(rt[:, 2 * g:2 * g + 2, :],
                                  _r8(src_t[g * 256:(g + 1) * 256, :], a=2))
                for cc in range(2 * g, 2 * g + 2):
                    eng = nc.gpsimd if cc % 2 else nc.vector
                    eng.tensor_add(xt[cc][:, hs], xt[cc][:, hs], rt[:, cc, :])

        def _load_half(pool, dram, width):
            ts = []
            for g in range(2):
                t = pool.tile([128, 4, width], bf16)
                nc.sync.dma_start(
                    t[:], _r8(dram[g * 512:(g + 1) * 512, :], a=4))
                ts.append(t)
            return ts

        def load_wq(l):
            t = _load_half(wqp, wqkv_d[l], 3 * QO)
            bvt = bvp.tile([128, QO], bf16)
            nc.sync.dma_start(bvt[:], bvb_d[l, :, :])
            ws = wsp.tile([1, 3 * QO], bf16, tag="wqs")
            nc.sync.dma_start(ws[:], wqs_d[l])
            return t, bvt, ws

        def load_w1(l):
            t = _load_half(w1p, w1_d[l], FL)
            ws = wsp.tile([1, FL], bf16, tag="w1s")
            nc.sync.dma_start(ws[:], w1s_d[l])
            return t, ws

        def load_w2(l):
            return _load_half(w2p, w2_d[l], C)

        ar_bufs = [None, None]
        head_pre = []

        # LM head helpers (normal orientation: out[tok, vocab])
        NVB = (VL + 511) // 512

        def head_load(vb):
            vn = min(512, VL - vb * 512)
            rhs_t = []
            for g in range(2):
                wt = hwp.tile([128, 4, 512], bf16)
                nc.sync.dma_start(
                    wt[:, :, 0:vn],
                    _r8(hw_d[g * 512:(g + 1) * 512,
                             vb * 512:vb * 512 + vn], a=4))
                rhs_t.append(wt)
            return rhs_t

        def head_block(vb, rhs_t, tccs):
            vn = min(512, VL - vb * 512)
            for tcc in tccs:
                to = (tcc % 4) * 128
                ph = psb.tile([128, 512], f32, tag="psb")
                for cc in range(NCH):
                    nc.tensor.matmul(ph[:, 0:vn],
                                     hf[tcc // 4][cc][:, to:to + 128],
                                     rhs_t[cc // 4][:, cc % 4, 0:vn],
                                     start=(cc == 0), stop=(cc == NCH - 1))
                so = sop.tile([128, 512], bf16, tag="so")
                if tcc % 2:
                    nc.vector.tensor_copy(so[:, 0:vn], ph[:, 0:vn])
                else:
                    nc.scalar.activation(so[:, 0:vn], ph[:, 0:vn], AF.Copy)
                nc.sync.dma_start(out_d[tcc * 128:(tcc + 1) * 128,
                                        vb * 512:vb * 512 + vn],
                                  so[:, 0:vn])


        # prologue: weights for layer 0 + LN1 of half 0
        wq_cur = load_wq(0)
        w1_cur = load_w1(0)
        w2_cur = load_w2(0)
        h1a_nm = ln_post(0, ln_stats(0))

        for l in range(L):
            (wq3, bvt, wqs), (w13, w1s), w23 = wq_cur, w1_cur, w2_cur

            qk_t = [qkp.tile([128, T], bf16, tag="qkt", name=f"qk{l}_{i}")
                    for i in range(4)]
            v_t = [[None] * HL for _ in range(NTC)]
            ag_bufs = [None, None]

            def qkv_qk(half, h1, nmr):
                hs = slice(half * TH, (half + 1) * TH)
                for oc in range(4):
                    p = psb.tile([128, TH], f32, tag="psb")
                    for cc in range(NCH):
                        nc.tensor.matmul(p[:],
                                         wq3[cc // 4][:, cc % 4,
                                                      oc * 128:(oc + 1) * 128],
                                         h1[cc][:],
                                         start=(cc == 0), stop=False)
                    nc.tensor.matmul(p[:],
                                     wqs[0:1, oc * 128:(oc + 1) * 128],
                                     nmr[:], start=False, stop=True)
                    if oc % 2:
                        nc.scalar.activation(
                            qk_t[oc][:, hs], p[:], AF.Identity,
                            bias=cols["bqk"][:, l * 4 + oc:l * 4 + oc + 1])
                    else:
                        nc.vector.tensor_scalar_add(
                            qk_t[oc][:, hs], p[:],
                            cols["bqk"][:, l * 4 + oc:l * 4 + oc + 1])

            def qkv_v(half, h1, nmr):
                # v chunks of this half (ones column fused for softmax denom)
                for tcc in range(4 * half, 4 * half + 4):
                    to = tcc * 128 - half * TH
                    pv = pss.tile([128, QO], f32, tag="pss")
                    for cc in range(NCH):
                        nc.tensor.matmul(pv[:], h1[cc][:, to:to + 128],
                                         wq3[cc // 4][:, cc % 4,
                                                      2 * QO:3 * QO],
                                         start=(cc == 0), stop=False)
                    nc.tensor.matmul(pv[:], nmr[0:1, to:to + 128],
                                     wqs[0:1, 2 * QO:3 * QO],
                                     start=False, stop=True)
                    for hh in range(HL):
                        vt = vp.tile([128, HD + 1], bf16)
                        nc.vector.memset(vt[:, HD:HD + 1], 1.0)
                        nc.vector.tensor_add(vt[:, 0:HD],
                                             pv[:, hh * HD:(hh + 1) * HD],
                                             bvt[:, hh * HD:(hh + 1) * HD])
                        v_t[tcc][hh] = vt

            def attn(half, mid=None, mid_at=1):
                """Head-pipelined attention + AllGather launch for a half."""
                y_sb = [yp.tile([128, TH], bf16, tag="y",
                                name=f"ysb{l}_{half}_{i}") for i in range(2)]
                nsi = 4 * half + 4

                def scores(hh):
                    qi, ro = hh // 2, (hh % 2) * 64
                    att = []
                    for si in range(nsi):
                        pa = psb.tile([128, TH], f32, tag="psb")
                        lhs = qk_t[2 + qi][ro:ro + 64,
                                           si * 128:(si + 1) * 128]
                        sc = max(si * 128 - half * TH, 0)
                        nc.tensor.matmul(pa[:, sc:TH], lhs,
                                         qk_t[qi][ro:ro + 64,
                                                  half * TH + sc:
                                                  (half + 1) * TH],
                                         start=True, stop=True)
                        ab = sbf.tile([128, TH], bf16, tag="sbf")
                        if sc:
                            nc.vector.memset(ab[:, 0:sc], 0.0)
                        nc.scalar.activation(ab[:, sc:TH], pa[:, sc:TH],
                                             AF.Exp, scale=float(SCALE))
                        if si >= 4 * half:  # diagonal block: causal mask
                            nc.vector.tensor_mul(ab[:, sc:sc + 128],
                                                 ab[:, sc:sc + 128], mask[:])
                        att.append(ab)
                    return att

                def av(hh, att):
                    py = pss.tile([HD + 1, TH], f32, tag="pss")
                    for qb in range(4):
                        qs = slice(qb * 128, (qb + 1) * 128)
                        last = 4 * half + qb
                        for si in range(last + 1):
                            nc.tensor.matmul(py[:, qs], v_t[si][hh][:],
                                             att[si][:, qs],
                                             start=(si == 0),
                                             stop=(si == last))
                    den_r = smb.tile([1, TH], bf16, tag="smb")
                    with nc.allow_low_precision(reason="softmax denom bf16"):
                        nc.vector.reciprocal(den_r[:], py[HD:HD + 1, :])
                    den_b = bcb.tile([64, TH], bf16, tag="bcb")
                    nc.gpsimd.partition_broadcast(den_b[:], den_r[:])
                    nc.vector.tensor_mul(
                        y_sb[hh // 2][(hh % 2) * 64:(hh % 2) * 64 + 64, :],
                        py[0:HD, :], den_b[:])

                prev = None
                for hh in range(HL):
                    att = scores(hh)
                    if prev is not None:
                        av(*prev)
                        if hh == mid_at and mid is not None:
                            mid()
                    prev = (hh, att)
                av(*prev)

                g_in = dr.tile([QO, TH], bf16, tag="gin")
                for i in range(2):
                    nc.sync.dma_start(g_in[i * 128:(i + 1) * 128, :],
                                      y_sb[i][:])
                g_out = dr.tile([C, TH], bf16, tag="gout")
                if collectives is True:
                    nc.gpsimd.collective_compute(
                        "AllGather", OP.bypass, replica_groups=GROUPS,
                        ins=[g_in.opt()], outs=[g_out.opt()])
                elif collectives == "local":
                    for q in range(TP):
                        nc.sync.dma_start(g_out[q * QO:(q + 1) * QO, :],
                                          g_in[:])
                ag_bufs[half] = (g_in, g_out)

            def ag_read(half):
                hs = slice(half * TH, (half + 1) * TH)
                g_in, g_out = ag_bufs[half]
                yt = sb8.tile([128, NCH, TH], bf16, tag="sb8")
                for g in range(4):
                    if collectives != "skip":
                        nc.sync.dma_start(
                            yt[:, 2 * g:2 * g + 2, :],
                            _r8(g_out[g * 256:(g + 1) * 256, :], a=2))
                    else:
                        for cc in range(2 * g, 2 * g + 2):
                            nc.sync.dma_start(
                                yt[:, cc, :],
                                g_in[(cc % 2) * 128:(cc % 2) * 128 + 128, :])
                    for cc in range(2 * g, 2 * g + 2):
                        eng = nc.gpsimd if cc % 2 else nc.vector
                        eng.tensor_add(xt[cc][:, hs], xt[cc][:, hs],
                                       yt[:, cc, :])

            def mlp1(half, h2, nmr2):
                a_t = []
                for fc in range(NCH):
                    pm = psb.tile([128, TH], f32, tag="psb")
                    for cc in range(NCH):
                        nc.tensor.matmul(pm[:],
                                         w13[cc // 4][:, cc % 4, fc * 128:(fc + 1) * 128],
                                         h2[cc][:],
                                         start=(cc == 0), stop=False)
                    nc.tensor.matmul(pm[:],
                                     w1s[0:1, fc * 128:(fc + 1) * 128],
                                     nmr2[:], start=False, stop=True)
                    ga = sbf.tile([128, TH], bf16, tag="sbf")
                    nc.scalar.activation(
                        ga[:], pm[:], AF.Gelu,
                        bias=cols["b1"][:, l * 8 + fc:l * 8 + fc + 1])
                    a_t.append(ga)
                return a_t

            def mlp2(half, a_t):
                mo = sb8.tile([128, NCH, TH], bf16, tag="sb8")
                for cc in range(NCH):
                    pm2 = psb.tile([128, TH], f32, tag="psb")
                    for fc in range(NCH):
                        nc.tensor.matmul(pm2[:],
                                         w23[fc // 4][:, fc % 4, cc * 128:(cc + 1) * 128],
                                         a_t[fc][:],
                                         start=(fc == 0), stop=(fc == NCH - 1))
                    b2c = cols["b2"][:, l * 8 + cc:l * 8 + cc + 1]
                    if cc % 2:
                        nc.vector.tensor_scalar_add(mo[:, cc, :], pm2[:], b2c)
                    else:
                        nc.scalar.activation(mo[:, cc, :], pm2[:],
                                             AF.Identity, bias=b2c)
                r_in = dr.tile([C, TH], bf16, tag="rin")
                r_out = dr.tile([C, TH], bf16, tag="rout")
                for g in range(4):
                    gs = slice(g * 256, (g + 1) * 256)
                    nc.sync.dma_start(_r8(r_in[gs, :], a=2),
                                      mo[:, 2 * g:2 * g + 2, :])
                    if collectives is True:
                        nc.gpsimd.collective_compute(
                            "AllReduce", OP.add, replica_groups=GROUPS,
                            ins=[r_in[gs, :].opt()], outs=[r_out[gs, :].opt()])
                    elif collectives == "local":
                        nc.sync.dma_start(r_out[gs, :], r_in[gs, :])
                ar_bufs[half] = (r_in, r_out)

            # schedule: LN chains / collective flights hide behind the other
            # half's PE phases; ln_stats PE matmuls are placed right after
            # phases that give their input dependencies time to resolve
            mid_state = {}

            h1a, nm1a = h1a_nm
            qkv_qk(0, h1a, nm1a)
            if ar_bufs[1] is not None:
                ar_read(l - 1, *ar_bufs[1], 1)
            qkv_v(0, h1a, nm1a)

            def mid0():
                mid_state["c1b"] = ln_chain(ln_stats(1))

            attn(0, mid=mid0, mid_at=1)
            h1b, nm1b = ln_norm(1, mid_state["c1b"])
            wq_cur = load_wq(l + 1) if l + 1 < L else None
            qkv_qk(1, h1b, nm1b)
            qkv_v(1, h1b, nm1b)
            ag_read(0)

            def mid1():
                mid_state["c2a"] = ln_chain(ln_stats(0))

            attn(1, mid=mid1)
            h2a, nm2a = ln_norm(0, mid_state["c2a"])
            w1_cur = load_w1(l + 1) if l + 1 < L else None
            a0 = mlp1(0, h2a, nm2a)
            ag_read(1)
            st2b = ln_stats(1)
            h2b, nm2b = ln_post(1, st2b)
            mlp2(0, a0)
            w2_cur = load_w2(l + 1) if l + 1 < L else None
            if l == L - 1:
                head_pre.append(head_load(0))
                head_pre.append(head_load(1))
            a1 = mlp1(1, h2b, nm2b)
            ar_read(l, *ar_bufs[0], 0)
            st1a = ln_stats(0)
            h1a_nm = ln_post(0, st1a, final=(l == L - 1))
            mlp2(1, a1)

        # first two vocab blocks' half-0 tokens cover the final LN of half 1
        hf = [h1a_nm[0], None]
        rhs0, rhs1 = head_pre
        head_block(0, rhs0, range(4))
        ar_read(L - 1, *ar_bufs[1], 1)
        stf = ln_stats(1)
        head_block(1, rhs1, range(4))
        hf[1] = ln_post(1, stf, final=True)[0]
        head_block(0, rhs0, range(4, NTC))
        head_block(1, rhs1, range(4, NTC))
        for vb in range(2, NVB):
            rhs_t = head_load(vb)
            head_block(vb, rhs_t, range(NTC))

def _prep_inputs(idx, tok_emb, pos_emb, ln1_w, ln1_b, wq, bq, wk, bk, wv, bv,
                 ln2_w, ln2_b, w1, b1, w2, b2, lnf_w, lnf_b, head_w):
    bf = ml_dtypes.bfloat16

    def cols128(a):  # [L, C] -> [128, L*8] per-partition column packing
        a = np.ascontiguousarray(a, np.float32)
        Lx = a.shape[0]
        return a.reshape(Lx, NCH, 128).transpose(2, 0, 1).reshape(128, Lx * NCH)

    # fold LN affine into the consuming projections:
    #   q = ((x-mu)*rstd) @ (ln1_w * wq) + (bq + ln1_b @ wq), etc.
    wq = np.asarray(wq, np.float32)
    wk = np.asarray(wk, np.float32)
    wv = np.asarray(wv, np.float32)
    w1 = np.asarray(w1, np.float32)
    bq = np.asarray(bq, np.float32) + np.einsum("lc,lcf->lf", ln1_b, wq)
    bk = np.asarray(bk, np.float32) + np.einsum("lc,lcf->lf", ln1_b, wk)
    bv = np.asarray(bv, np.float32) + np.einsum("lc,lcf->lf", ln1_b, wv)
    b1 = np.asarray(b1, np.float32) + np.einsum("lc,lcf->lf", ln2_b, w1)
    wq = ln1_w[:, :, None] * wq
    wk = ln1_w[:, :, None] * wk
    wv = ln1_w[:, :, None] * wv
    w1 = ln2_w[:, :, None] * w1

    mask = np.zeros((128, 128), np.float32)
    p, t = np.meshgrid(np.arange(128), np.arange(128), indexing="ij")
    mask[p <= t] = 1.0
    in_maps = []
    shard_cache = {}
    x0s = [np.ascontiguousarray(
        (tok_emb[np.asarray(idx[g], np.int64)] + pos_emb[0]).T, np.float32)
        for g in range(B)]
    for c in range(8):
        g, j = c // 4, c % 4
        if j in shard_cache:
            m = dict(shard_cache[j])
            m["x0t"] = x0s[g]
            in_maps.append(m)
            continue
        m = {
            "wqkv": np.ascontiguousarray(np.concatenate(
                [wq[:, :, j * QO:(j + 1) * QO], wk[:, :, j * QO:(j + 1) * QO],
                 wv[:, :, j * QO:(j + 1) * QO]], axis=2)).astype(bf),
            "w1": np.ascontiguousarray(w1[:, :, j * FL:(j + 1) * FL]).astype(bf),
            "w2": np.ascontiguousarray(w2[:, j * FL:(j + 1) * FL, :]).astype(bf),
            "hw": np.ascontiguousarray(
                lnf_w[:, None] * head_w[:, j * VL:(j + 1) * VL]).astype(bf),

            "bqk": np.ascontiguousarray(np.stack(
                [bq[:, j * QO:(j + 1) * QO].reshape(L, 2, 128),
                 bk[:, j * QO:(j + 1) * QO].reshape(L, 2, 128)],
                axis=1).reshape(L * 4, 128).T, np.float32),
            "bvb": np.ascontiguousarray(np.broadcast_to(
                bv[:, None, j * QO:(j + 1) * QO],
                (L, 128, QO)).astype(bf)),
            "wqs": np.concatenate(
                [wq[:, :, j * QO:(j + 1) * QO].sum(axis=1),
                 wk[:, :, j * QO:(j + 1) * QO].sum(axis=1),
                 wv[:, :, j * QO:(j + 1) * QO].sum(axis=1)],
                axis=1)[:, None, :].astype(bf),
            "w1s": w1[:, :, j * FL:(j + 1) * FL].sum(axis=1)[:, None, :]
                .astype(bf),
            "b1c": cols128(b1[:, j * FL:(j + 1) * FL]),
            "b2c": cols128(b2 if j == 0 else np.zeros_like(
                np.asarray(b2, np.float32))),
            "lnfw": cols128(lnf_w[None]), "lnfb": cols128(lnf_b[None]),
            "mask": mask.astype(bf),
        }
        m["x0t"] = x0s[g]
        shard_cache[j] = m
        in_maps.append(m)
    return in_maps


def kernel(**inputs):
    if "nc" not in _STATE:
        _STATE["nc"] = _build()
    nc = _STATE["nc"]
    ins = {k: np.asarray(v) for k, v in inputs.items()}
    in_maps = _prep_inputs(**ins)
    res = bass_utils.run_bass_kernel_spmd(nc, in_maps, core_ids=list(range(8)))
    outs = res.results
    # lnf_b's contribution to the logits is a per-vocab constant, added here
    hb = (np.asarray(ins["lnf_b"], np.float32)
          @ np.asarray(ins["head_w"], np.float32))
    full = np.empty((B, T, V), np.float32)
    for c in range(8):
        g, j = c // 4, c % 4
        full[g, :, j * VL:(j + 1) * VL] = (
            np.asarray(outs[c]["out"], np.float32) + hb[j * VL:(j + 1) * VL])
    return full
